# revision 25
# baseline (speedup 1.0000x reference)
"""Trainium2 Bass kernel for nn_Attention_v1_13735305413199.

Fully-fused on-device pipeline, one batch per core (4 of 8 cores used).
Per core: conv1x1(w_in half)+conv1x1(w_qkv)+dwconv3x3 as 9 tap-folded
matmuls (both branches), l2-norm + gram + masked softmax attention,
per-channel cross-branch combine (s1@o2 + s2@o1) via PE transposes, and
the final w_out projection — so only the final [192, 16384] fp16 tensor
crosses the (slow) axon tunnel per core, instead of qkv [576,16384].
"""

import os
import numpy as np
import ml_dtypes

import jax

# Persistent XLA compilation cache: run_bass_kernel_spmd rebuilds its jit
# closure every call, so the in-memory jit cache always misses; the disk
# cache (keyed on HLO) turns the per-call re-compile into a ~50ms hit.
jax.config.update("jax_compilation_cache_dir", "/root/.jax_kernel_cache")
jax.config.update("jax_persistent_cache_min_entry_size_bytes", -1)
jax.config.update("jax_persistent_cache_min_compile_time_secs", 0)

import concourse.bacc as bacc
import concourse.bass as bass
import concourse.masks as masks
import concourse.mybir as mybir
import concourse.tile as tile
from concourse import bass_utils

HEADS = 8
DIM = 192
B, H, W = 4, 128, 128
HW = H * W
C3 = 3 * DIM  # 576
PH = H + 2  # 130
NCORES = int(os.environ.get("KCORES", "4"))

F32 = mybir.dt.float32
F16 = mybir.dt.float16
BF16 = mybir.dt.bfloat16
AX = mybir.AxisListType
ALU = mybir.AluOpType
ACTF = mybir.ActivationFunctionType

_cache = {}


def _build_program():
    STAGE = int(os.environ.get("KSTAGE", "4"))
    key = ("nc", STAGE)
    if key in _cache:
        return _cache[key]
    nc = bacc.Bacc(
        "TRN2",
        target_bir_lowering=False,
        debug=False,
        enable_asserts=False,
        num_devices=NCORES,
    )
    xp_d = nc.dram_tensor("xp", [DIM, PH * PH], mybir.dt.int8, kind="ExternalInput")
    xs_d = nc.dram_tensor("xs", [DIM, 1], F32, kind="ExternalInput")
    win_d = nc.dram_tensor("win", [2 * DIM, DIM], F16, kind="ExternalInput")
    wq_d = nc.dram_tensor("wq", [2, DIM, C3], F16, kind="ExternalInput")
    kdw_d = nc.dram_tensor("kdw", [2, 9, C3], F16, kind="ExternalInput")
    mask_d = nc.dram_tensor("mask", [DIM, DIM], F32, kind="ExternalInput")
    trow_d = nc.dram_tensor("trow", [DIM, 1], F32, kind="ExternalInput")
    woT_d = nc.dram_tensor("woT", [DIM, DIM], F16, kind="ExternalInput")
    out_d = nc.dram_tensor("out", [DIM, HW], mybir.dt.int8, kind="ExternalOutput")
    osc_d = nc.dram_tensor("osc", [DIM, HW // 512], F32, kind="ExternalOutput")
    stage_d = nc.dram_tensor("stage", [DIM, HW], F16, kind="Internal")

    KCH = [(0, 128), (128, 64)]
    MCH = [(0, 128), (128, 128), (256, 128), (384, 128), (512, 64)]
    TAPS = [(di, dj) for di in range(3) for dj in range(3)]
    NT = HW // 512  # 32

    with tile.TileContext(nc) as tc:
        with (
            tc.tile_pool(name="const", bufs=1) as cpool,
            tc.tile_pool(name="ospill", bufs=1, space="DRAM") as dpool,
        ):
            ident = cpool.tile([128, 128], F16, tag="idf16")
            masks.make_identity(nc, ident[:, :])
            identb = cpool.tile([128, 128], BF16, tag="idbf16")
            masks.make_identity(nc, identb[:, :])
            identf = cpool.tile([128, 128], F32, tag="idf32")
            masks.make_identity(nc, identf[:, :])
            ones_row = cpool.tile([1, 128], F32, tag="ones")
            nc.vector.memset(ones_row[:, :], 1.0)
            ones16 = cpool.tile([1, 128], F16, tag="ones16")
            nc.vector.memset(ones16[:, :], 1.0)
            mask_a = cpool.tile([128, DIM], F32, tag="mska")
            nc.sync.dma_start(mask_a[:, :], mask_d[0:128, :])
            mask_b = cpool.tile([64, DIM], F32, tag="mskb")
            nc.sync.dma_start(mask_b[:, :], mask_d[128:192, :])
            trow_a = cpool.tile([128, 1], F32, tag="tra")
            nc.sync.dma_start(trow_a[:, :], trow_d[0:128, :])
            trow_b = cpool.tile([64, 1], F32, tag="trb")
            nc.sync.dma_start(trow_b[:, :], trow_d[128:192, :])
            woT_a = cpool.tile([128, DIM], F16, tag="woa")
            nc.sync.dma_start(woT_a[:, :], woT_d[0:128, :])
            woT_b = cpool.tile([64, DIM], F16, tag="wob")
            nc.sync.dma_start(woT_b[:, :], woT_d[128:192, :])

            o_sp = [
                dpool.tile([DIM, HW], F16, tag=f"osp{br}", name=f"osp{br}")
                for br in range(2)
            ]

            # ============ branch phases: conv + gram + attention ============
            with (
                tc.tile_pool(name="xin", bufs=1) as xin_pool,
                tc.tile_pool(name="wts", bufs=1) as wt_pool,
                tc.tile_pool(name="vkeep", bufs=1) as v_pool,
                tc.tile_pool(name="qksb", bufs=2) as qk_pool,
                tc.tile_pool(name="qkt", bufs=2) as qkt_pool,
                tc.tile_pool(name="small", bufs=1) as sm_pool,
                tc.tile_pool(name="att", bufs=1) as at_pool,
                tc.tile_pool(name="ost", bufs=2) as ost_pool,
                tc.tile_pool(name="psmm", bufs=2, space="PSUM") as ps_mm,
                tc.tile_pool(name="pst", bufs=2, space="PSUM") as ps_t,
                tc.tile_pool(name="psg", bufs=1, space="PSUM") as ps_g,
            ):
                xsb = []
                for kc, (ko, kw) in enumerate(KCH):
                    # int8 staging borrows the v-tile slots (free until conv out)
                    tq = v_pool.tile(
                        [128, PH * PH], mybir.dt.int8,
                        tag=("va" if kc == 0 else "vb"), name=f"xq{kc}",
                    )
                    nc.sync.dma_start(tq[:kw, :], xp_d[ko : ko + kw, :])
                    xsc = xin_pool.tile([128, 1], F32, tag=f"xs{kc}", name=f"xs{kc}")
                    nc.sync.dma_start(xsc[:kw, :], xs_d[ko : ko + kw, :])
                    t = xin_pool.tile([128, PH * PH], BF16, tag=f"x{kc}")
                    nc.vector.tensor_scalar_mul(t[:kw, :], tq[:kw, :], xsc[:kw, :])
                    xsb.append(t.rearrange("p (h w) -> p h w", h=PH))

                for br in range(2):
                    # ---- fold conv weights on device:
                    # wt[kc][k, t*C3+o] = (w_qkv @ w_in_half).T[k, o] * kdw[o, t]
                    wsb = []
                    for kc in range(2):
                        t = wt_pool.tile([128, 9 * C3], BF16, tag=f"w{kc}")
                        wsb.append(t)
                    wina = wt_pool.tile([128, DIM], F16, tag="wina")
                    nc.sync.dma_start(wina[:, :], win_d[br * DIM : br * DIM + 128, :])
                    winb = wt_pool.tile([64, DIM], F16, tag="winb")
                    nc.sync.dma_start(
                        winb[:, :], win_d[br * DIM + 128 : br * DIM + DIM, :]
                    )
                    wqa = wt_pool.tile([128, C3], F16, tag="wqa")
                    nc.sync.dma_start(wqa[:, :], wq_d[br, 0:128, :])
                    wqb = wt_pool.tile([64, C3], F16, tag="wqb")
                    nc.sync.dma_start(wqb[:, :], wq_d[br, 128:192, :])
                    kdw_sb = wt_pool.tile([1, 9 * C3], F16, tag="kdw")
                    nc.sync.dma_start(
                        kdw_sb[:, :], kdw_d[br].rearrange("t c -> (t c)").unsqueeze(0)
                    )
                    weff_a = wt_pool.tile([128, C3], F32, tag="weffa")
                    weff_b = wt_pool.tile([64, C3], F32, tag="weffb")
                    for ci, (weff_t, cw) in enumerate(((weff_a, 128), (weff_b, 64))):
                        for oo, ow in ((0, 512), (512, 64)):
                            fps = ps_mm.tile([128, 512], F32, tag="mm")
                            nc.tensor.matmul(
                                fps[:cw, :ow],
                                wina[:, ci * 128 : ci * 128 + cw],
                                wqa[:, oo : oo + ow],
                                start=True, stop=False,
                            )
                            nc.tensor.matmul(
                                fps[:cw, :ow],
                                winb[:64, ci * 128 : ci * 128 + cw],
                                wqb[:64, oo : oo + ow],
                                start=False, stop=True,
                            )
                            nc.vector.tensor_copy(weff_t[:cw, oo : oo + ow], fps[:cw, :ow])
                    for t in range(9):
                        bck = wt_pool.tile([128, C3], F32, tag="bck")
                        for oo, ow in ((0, 512), (512, 64)):
                            fps = ps_mm.tile([128, 512], F32, tag="mm")
                            nc.tensor.matmul(
                                fps[:, :ow],
                                ones16[0:1, :],
                                kdw_sb[0:1, t * C3 + oo : t * C3 + oo + ow],
                                start=True, stop=True,
                            )
                            nc.vector.tensor_copy(bck[:, oo : oo + ow], fps[:, :ow])
                        nc.vector.tensor_tensor(
                            wsb[0][:, t * C3 : (t + 1) * C3],
                            weff_a[:, :], bck[:, :], ALU.mult,
                        )
                        nc.vector.tensor_tensor(
                            wsb[1][:64, t * C3 : (t + 1) * C3],
                            weff_b[:64, :], bck[:64, :], ALU.mult,
                        )
                    va = v_pool.tile([128, HW], F16, tag="va")
                    vb = v_pool.tile([64, HW], F16, tag="vb")
                    nsq = [
                        sm_pool.tile([128, NT], F32, tag=f"nsq{i}", name=f"nsq{i}")
                        for i in range(3)
                    ]
                    sq_scr = sm_pool.tile([128, 512], F32, tag="sqscr")
                    ga_ps = ps_g.tile([128, DIM], F32, tag="ga")
                    gb_ps = ps_g.tile([64, DIM], F32, tag="gb")

                    qk_sb = [None, None, None]
                    for nt in range(NT):
                        h0 = nt * 4
                        for mi, (mo, mw) in enumerate(MCH):
                            ps = ps_mm.tile([128, 512], F32, tag="mm")
                            idx = 0
                            for ti, (di, dj) in enumerate(TAPS):
                                for kc, (ko, kw) in enumerate(KCH):
                                    nc.tensor.matmul(
                                        ps[:mw, :],
                                        wsb[kc][:kw, ti * C3 + mo : ti * C3 + mo + mw],
                                        xsb[kc][:kw, h0 + di : h0 + di + 4, dj : dj + 128],
                                        start=(idx == 0),
                                        stop=(idx == 17),
                                    )
                                    idx += 1
                            if mi < 3:
                                sb = qk_pool.tile([128, 512], F16, tag=f"qk{mi}")
                                nc.scalar.copy(sb[:mw, :], ps[:mw, :])
                                qk_sb[mi] = sb
                                # row sum-of-squares for q/k l2 norms
                                nc.vector.scalar_tensor_tensor(
                                    out=sq_scr[:mw, :],
                                    in0=sb[:mw, :],
                                    scalar=1.0,
                                    in1=sb[:mw, :],
                                    op0=ALU.mult,
                                    op1=ALU.mult,
                                    accum_out=nsq[mi][:mw, nt : nt + 1],
                                )
                            elif mi == 3:
                                nc.scalar.copy(
                                    va[:, nt * 512 : (nt + 1) * 512], ps[:128, :]
                                )
                            else:
                                nc.scalar.copy(
                                    vb[:64, nt * 512 : (nt + 1) * 512], ps[:64, :]
                                )
                        # transpose q,k 128-blocks and accumulate gram
                        for sub in range(STAGE >= 2 and 4 or 0):
                            qkt = qkt_pool.tile([128, 384], F16, tag="qkt")
                            for c in range(3):
                                tps = ps_t.tile([128, 128], F16, tag="t")
                                nc.tensor.transpose(
                                    tps[:, :],
                                    qk_sb[c][:, sub * 128 : (sub + 1) * 128],
                                    ident[:, :],
                                )
                                nc.vector.tensor_copy(
                                    qkt[:, c * 128 : (c + 1) * 128], tps[:, :]
                                )
                            first = nt == 0 and sub == 0
                            last = nt == NT - 1 and sub == 3
                            nc.tensor.matmul(
                                ga_ps[:, :],
                                qkt[:, 0:128],
                                qkt[:, 192:384],
                                start=first,
                                stop=last,
                                skip_group_check=True,
                            )
                            nc.tensor.matmul(
                                gb_ps[:, :],
                                qkt[:, 128:192],
                                qkt[:, 192:384],
                                start=first,
                                stop=last,
                                skip_group_check=True,
                            )

                    # ---- norms -> inverse, gram normalize, masked softmax ----
                    if STAGE < 3:
                        continue
                    inv = []
                    for i in range(3):
                        s = sm_pool.tile([128, 1], F32, tag=f"inv{i}")
                        nc.vector.reduce_sum(s[:, :], nsq[i][:, :], axis=AX.X)
                        nc.scalar.sqrt(s[:, :], s[:, :])
                        nc.vector.tensor_scalar_max(s[:, :], s[:, :], 1e-12)
                        nc.vector.reciprocal(s[:, :], s[:, :])
                        inv.append(s)
                    # invnq * temperature (per-partition scalars for rows)
                    iqt_a = sm_pool.tile([128, 1], F32, tag="iqta")
                    nc.vector.tensor_tensor(
                        iqt_a[:, :], inv[0][:, :], trow_a[:, :], ALU.mult
                    )
                    iqt_b = sm_pool.tile([64, 1], F32, tag="iqtb")
                    nc.vector.tensor_tensor(
                        iqt_b[:, :], inv[1][0:64, :], trow_b[:, :], ALU.mult
                    )
                    # invnk as a broadcast [128, 192] via transpose + outer product
                    krow = sm_pool.tile([1, DIM], F32, tag="krow")
                    tps = ps_t.tile([128, 192], F32, tag="t")
                    nc.tensor.transpose(tps[0:1, 0:128], inv[1][:, :], identf[:, :])
                    nc.vector.tensor_copy(krow[:, 0:64], tps[0:1, 64:128])
                    tps = ps_t.tile([128, 192], F32, tag="t")
                    nc.tensor.transpose(tps[0:1, 0:128], inv[2][:, :], identf[:, :])
                    nc.vector.tensor_copy(krow[:, 64:192], tps[0:1, 0:128])
                    bc_ps = ps_t.tile([128, 192], F32, tag="t")
                    nc.tensor.matmul(
                        bc_ps[:, :], ones_row[0:1, :], krow[0:1, :], start=True, stop=True
                    )
                    bc = sm_pool.tile([128, DIM], F32, tag="bc")
                    nc.vector.tensor_copy(bc[:, :], bc_ps[:, :])

                    att = []
                    for (g_ps, mw, iqt, msk) in (
                        (ga_ps, 128, iqt_a, mask_a),
                        (gb_ps, 64, iqt_b, mask_b),
                    ):
                        g = sm_pool.tile([mw, DIM], F32, tag=f"g{mw}")
                        nc.vector.scalar_tensor_tensor(
                            out=g[:, :],
                            in0=g_ps[:, :],
                            scalar=iqt[:, :],
                            in1=bc[:mw, :],
                            op0=ALU.mult,
                            op1=ALU.mult,
                        )
                        nc.vector.tensor_tensor(g[:, :], g[:, :], msk[:, :], ALU.add)
                        negm = sm_pool.tile([mw, 1], F32, tag=f"ngm{mw}")
                        nc.vector.reduce_max(negm[:, :], g[:, :], axis=AX.X, negate=True)
                        ssum = sm_pool.tile([mw, 1], F32, tag=f"ssm{mw}")
                        nc.scalar.activation(
                            g[:, :], g[:, :], ACTF.Exp,
                            bias=negm[:, :], accum_out=ssum[:, :],
                        )
                        nc.vector.reciprocal(ssum[:, :], ssum[:, :])
                        a_sb = sm_pool.tile([mw, DIM], F16, tag=f"a{mw}")
                        nc.vector.tensor_scalar_mul(a_sb[:, :], g[:, :], ssum[:, :])
                        att.append(a_sb)

                    # A^T tiles: ATa = A^T[d 0:128, c 0:192], ATb = A^T[d 128:192, :]
                    at_a = at_pool.tile([128, DIM], F16, tag="ata")
                    at_b = at_pool.tile([64, DIM], F16, tag="atb")
                    for (src, d0, dw, c0, cw, dst) in (
                        (att[0], 0, 128, 0, 128, at_a),
                        (att[1], 0, 128, 128, 64, at_a),
                        (att[0], 128, 64, 0, 128, at_b),
                        (att[1], 128, 64, 128, 64, at_b),
                    ):
                        tps = ps_t.tile([128, 128], F16, tag="t")
                        nc.tensor.transpose(
                            tps[:dw, :cw], src[:cw, d0 : d0 + dw], ident[:cw, :cw]
                        )
                        nc.vector.tensor_copy(dst[:dw, c0 : c0 + cw], tps[:dw, :cw])

                    # ---- attn @ v -> o (fp16) spilled to DRAM ----
                    for nt in range(NT):
                        sl = slice(nt * 512, (nt + 1) * 512)
                        oa_ps = ps_mm.tile([128, 512], F32, tag="mm")
                        nc.tensor.matmul(
                            oa_ps[:, :], at_a[:, 0:128], va[:, sl], start=True, stop=False
                        )
                        nc.tensor.matmul(
                            oa_ps[:, :], at_b[:, 0:128], vb[:64, sl], start=False, stop=True
                        )
                        ob_ps = ps_mm.tile([64, 512], F32, tag="mmb")
                        nc.tensor.matmul(
                            ob_ps[:, :], at_a[:, 128:192], va[:, sl], start=True, stop=False
                        )
                        nc.tensor.matmul(
                            ob_ps[:, :], at_b[:, 128:192], vb[:64, sl], start=False, stop=True
                        )
                        os_a = ost_pool.tile([128, 512], F16, tag="osa")
                        nc.scalar.copy(os_a[:, :], oa_ps[:, :])
                        nc.sync.dma_start(o_sp[br][0:128, sl], os_a[:, :])
                        os_b = ost_pool.tile([64, 512], F16, tag="osb")
                        nc.scalar.copy(os_b[:, :], ob_ps[:, :])
                        nc.sync.dma_start(o_sp[br][128:192, sl], os_b[:, :])

            if STAGE < 4:
                # debug fallback: emit zeros so the output tensor is written
                with tc.tile_pool(name="dbg", bufs=1) as dbg_pool:
                    dbg_a = dbg_pool.tile([128, HW], F16, tag="dbga")
                    nc.vector.memset(dbg_a[:, :], 0.0)
                    nc.sync.dma_start(out_d[0:128, :], dbg_a[:, :])
                    nc.sync.dma_start(out_d[128:192, :], dbg_a[0:64, :])

            # ============ cross-branch combine + output projection ============
            if STAGE >= 4:
             with (
                tc.tile_pool(name="otp", bufs=1) as ot_pool,
                tc.tile_pool(name="otc", bufs=1) as otc_pool,
                tc.tile_pool(name="ld6", bufs=2) as ld_pool,
                tc.tile_pool(name="sm6", bufs=2) as sm6_pool,
                tc.tile_pool(name="rhs7", bufs=2) as rhs_pool,
                tc.tile_pool(name="cn8", bufs=2) as cn_pool,
                tc.tile_pool(name="fo8", bufs=2) as fo_pool,
                tc.tile_pool(name="pst6", bufs=3, space="PSUM") as ps_t6,
                tc.tile_pool(name="psc7", bufs=2, space="PSUM") as ps_c7,
                tc.tile_pool(name="psf8", bufs=1, space="PSUM") as ps_f8,
            ):
                otc = otc_pool.tile([128, DIM * 128], F16, tag="otc")
                otc_v = otc.rearrange("p (c h) -> p c h", h=128)

                for half in range(2):
                    r0 = half * 96
                    ots = []
                    oto = []
                    for br in range(2):
                        ts = ot_pool.tile([128, 96 * 128], F16, tag=f"ots{br}")
                        to = ot_pool.tile([128, 96 * 128], F16, tag=f"oto{br}")
                        ots.append(ts.rearrange("p (c h) -> p c h", h=128))
                        oto.append(to.rearrange("p (c h) -> p c h", h=128))

                    for br in range(2):
                        for nt in range(NT):
                            sl = slice(nt * 512, (nt + 1) * 512)
                            ld = ld_pool.tile([96, 512], F16, tag=f"ld{br}")
                            nc.sync.dma_start(ld[:, :], o_sp[br][r0 : r0 + 96, sl])
                            ld_v = ld.rearrange("p (s w) -> p s w", w=128)
                            # segmented softmax over w (4 windows of 128)
                            negm = sm6_pool.tile([96, 4], F32, tag="negm")
                            nc.vector.reduce_max(
                                negm[:, :], ld_v[:, :, :], axis=AX.X, negate=True
                            )
                            ex = sm6_pool.tile([96, 512], F32, tag="ex")
                            ex_v = ex.rearrange("p (s w) -> p s w", w=128)
                            nc.vector.tensor_tensor(
                                ex_v[:, :, :],
                                ld_v[:, :, :],
                                negm[:, :].unsqueeze(-1).broadcast_to([96, 4, 128]),
                                ALU.add,
                            )
                            nc.scalar.activation(ex[:, :], ex[:, :], ACTF.Exp)
                            ssum = sm6_pool.tile([96, 4], F32, tag="ssum")
                            nc.vector.reduce_sum(
                                ssum[:, :], ex_v[:, :, :], axis=AX.X
                            )
                            nc.vector.reciprocal(ssum[:, :], ssum[:, :])
                            s16 = sm6_pool.tile([96, 512], F16, tag="s16")
                            s16_v = s16.rearrange("p (s w) -> p s w", w=128)
                            nc.vector.tensor_tensor(
                                s16_v[:, :, :],
                                ex_v[:, :, :],
                                ssum[:, :].unsqueeze(-1).broadcast_to([96, 4, 128]),
                                ALU.mult,
                            )
                            for sub in range(4):
                                h = nt * 4 + sub
                                wsl = slice(sub * 128, (sub + 1) * 128)
                                tps = ps_t6.tile([128, 128], F16, tag="t6")
                                nc.tensor.transpose(
                                    tps[:, 0:96], s16[:, wsl], ident[0:96, 0:96]
                                )
                                nc.vector.tensor_copy(
                                    ots[br][:, :, h : h + 1],
                                    tps[:, 0:96].unsqueeze(-1),
                                )
                                tps = ps_t6.tile([128, 128], F16, tag="t6")
                                nc.tensor.transpose(
                                    tps[:, 0:96], ld[:, wsl], ident[0:96, 0:96]
                                )
                                nc.vector.tensor_copy(
                                    oto[br][:, :, h : h + 1],
                                    tps[:, 0:96].unsqueeze(-1),
                                )

                    # per-channel combine: comb[c] = s1[c] @ o2[c] + s2[c] @ o1[c]
                    for ci in range(96):
                        rhs = []
                        for br in (1, 0):  # rhs for term1 is o2, term2 is o1
                            tps = ps_t6.tile([128, 128], F16, tag="t6")
                            nc.tensor.transpose(
                                tps[:, :],
                                oto[br][:, ci : ci + 1, :],
                                ident[:, :],
                            )
                            r = rhs_pool.tile([128, 128], F16, tag=f"rhs{br}")
                            nc.vector.tensor_copy(r[:, :], tps[:, :])
                            rhs.append(r)
                        cps = ps_c7.tile([128, 128], F32, tag="comb")
                        nc.tensor.matmul(
                            cps[:, :], ots[0][:, ci : ci + 1, :], rhs[0][:, :],
                            start=True, stop=False,
                        )
                        nc.tensor.matmul(
                            cps[:, :], ots[1][:, ci : ci + 1, :], rhs[1][:, :],
                            start=False, stop=True,
                        )
                        comb = rhs_pool.tile([128, 128], F16, tag="comb16")
                        nc.scalar.copy(comb[:, :], cps[:, :])
                        tps = ps_t6.tile([128, 128], F16, tag="t6")
                        nc.tensor.transpose(tps[:, :], comb[:, :], ident[:, :])
                        nc.vector.tensor_copy(
                            otc_v[:, r0 + ci : r0 + ci + 1, :],
                            tps[:, :].unsqueeze(1),
                        )

                # ---- back to channel-major + w_out projection ----
                amx_a = cn_pool.tile([128, NT], F32, tag="amxa", bufs=1)
                amx_b = cn_pool.tile([64, NT], F32, tag="amxb", bufs=1)
                for nt in range(NT):
                    sl = slice(nt * 512, (nt + 1) * 512)
                    cn_a = cn_pool.tile([128, 512], F16, tag="cna")
                    cn_b = cn_pool.tile([64, 512], F16, tag="cnb")
                    for sub in range(4):
                        h = nt * 4 + sub
                        wsl = slice(sub * 128, (sub + 1) * 128)
                        tps = ps_t6.tile([128, 128], F16, tag="t6")
                        nc.tensor.transpose(
                            tps[:, :], otc_v[:, 0:128, h : h + 1], ident[:, :]
                        )
                        nc.vector.tensor_copy(cn_a[:, wsl], tps[:, :])
                        tps = ps_t6.tile([128, 128], F16, tag="t6")
                        nc.tensor.transpose(
                            tps[:64, :], otc_v[:, 128:192, h : h + 1], ident[:, :]
                        )
                        nc.vector.tensor_copy(cn_b[:, wsl], tps[:64, :])
                    fa_ps = ps_f8.tile([128, 512], F32, tag="fa")
                    nc.tensor.matmul(
                        fa_ps[:, :], woT_a[:, 0:128], cn_a[:, :], start=True, stop=False
                    )
                    nc.tensor.matmul(
                        fa_ps[:, :], woT_b[:, 0:128], cn_b[:, :], start=False, stop=True
                    )
                    fb_ps = ps_f8.tile([64, 512], F32, tag="fb")
                    nc.tensor.matmul(
                        fb_ps[:, :], woT_a[:, 128:192], cn_a[:, :], start=True, stop=False
                    )
                    nc.tensor.matmul(
                        fb_ps[:, :], woT_b[:, 128:192], cn_b[:, :], start=False, stop=True
                    )
                    fo_a = fo_pool.tile([128, 512], F16, tag="foa")
                    nc.scalar.copy(fo_a[:, :], fa_ps[:, :])
                    nc.sync.dma_start(stage_d[0:128, sl], fo_a[:, :])
                    nc.vector.reduce_max(
                        amx_a[:, nt : nt + 1], fa_ps[:, :], axis=AX.X,
                        apply_absolute_value=True,
                    )
                    fo_b = fo_pool.tile([64, 512], F16, tag="fob")
                    nc.scalar.copy(fo_b[:, :], fb_ps[:, :])
                    nc.sync.dma_start(stage_d[128:192, sl], fo_b[:, :])
                    nc.vector.reduce_max(
                        amx_b[:, nt : nt + 1], fb_ps[:, :], axis=AX.X,
                        apply_absolute_value=True,
                    )

                # quantize staged fp16 output to int8, per (channel, 512-block)
                for amx, r0, rw in ((amx_a, 0, 128), (amx_b, 128, 64)):
                    am = cn_pool.tile([rw, NT], F32, tag=f"am{r0}", bufs=1, name=f"am{r0}")
                    nc.vector.tensor_scalar_max(am[:, :], amx[:, :], 1e-30)
                    sc = cn_pool.tile([rw, NT], F32, tag=f"sc{r0}", bufs=1, name=f"sc{r0}")
                    nc.vector.tensor_scalar_mul(sc[:, :], am[:, :], 1.0 / 127.0)
                    nc.sync.dma_start(osc_d[r0 : r0 + rw, :], sc[:, :])
                    rq = cn_pool.tile([rw, NT], F32, tag=f"rq{r0}", bufs=1, name=f"rq{r0}")
                    nc.vector.reciprocal(rq[:, :], am[:, :])
                    nc.vector.tensor_scalar_mul(rq[:, :], rq[:, :], 127.0)
                    for nt in range(NT):
                        sl = slice(nt * 512, (nt + 1) * 512)
                        ld = fo_pool.tile([rw, 512], F16, tag=f"qld{r0}", name=f"qld{r0}")
                        nc.sync.dma_start(ld[:, :], stage_d[r0 : r0 + rw, sl])
                        qt = fo_pool.tile([rw, 512], mybir.dt.int8, tag=f"qt{r0}", name=f"qt{r0}")
                        # HW fp->int8 convert rounds to nearest (sim truncates;
                        # hardware is truth -- see pitfalls.md)
                        nc.vector.tensor_scalar_mul(qt[:, :], ld[:, :], rq[:, nt : nt + 1])
                        nc.sync.dma_start(out_d[r0 : r0 + rw, sl], qt[:, :])

    nc.compile()
    _cache[key] = nc
    return nc


def _host_prep(x, w_in, w_qkv1, w_dw1, w_qkv2, w_dw2, temperature, w_out):
    win = np.ascontiguousarray(w_in, np.float16)  # [384, 192]
    wq = np.stack(
        [np.ascontiguousarray(w_qkv1.T), np.ascontiguousarray(w_qkv2.T)]
    ).astype(np.float16)  # [2, 192, 576]
    kdw = np.stack(
        [
            np.ascontiguousarray(w_dw1.reshape(C3, 9).T),
            np.ascontiguousarray(w_dw2.reshape(C3, 9).T),
        ]
    ).astype(np.float16)  # [2, 9, 576]

    amax = np.maximum(np.abs(x).max(axis=(2, 3)), 1e-30)  # [B, DIM]
    xscale = (amax / 127.0).astype(np.float32)
    xq = np.rint(x / xscale[:, :, None, None]).astype(np.int8)
    xpad = np.zeros((B, DIM, PH, PH), np.int8)
    xpad[:, :, 1 : 1 + H, 1 : 1 + W] = xq
    xpad = xpad.reshape(B, DIM, PH * PH)

    hd = np.arange(DIM) // (DIM // HEADS)
    mask = np.where(hd[:, None] == hd[None, :], 0.0, -1e9).astype(np.float32)
    trow = np.repeat(temperature.reshape(HEADS), DIM // HEADS).astype(np.float32)
    woT = np.ascontiguousarray(w_out.T).astype(np.float16)
    return (xpad, xscale), (win, wq, kdw), mask, trow.reshape(DIM, 1), woT


def kernel(x, w_in, w_qkv1, w_dw1, w_qkv2, w_dw2, temperature, w_out):
    x = np.asarray(x, np.float32)
    xpad, wt, mask, trow, woT = _host_prep(
        x,
        np.asarray(w_in, np.float32),
        np.asarray(w_qkv1, np.float32),
        np.asarray(w_dw1, np.float32),
        np.asarray(w_qkv2, np.float32),
        np.asarray(w_dw2, np.float32),
        np.asarray(temperature, np.float32),
        np.asarray(w_out, np.float32),
    )

    nc = _build_program()
    win, wq, kdw = wt
    xpad, xscale = xpad
    in_maps = [
        {"xp": xpad[b % B], "xs": xscale[b % B].reshape(DIM, 1),
         "win": win, "wq": wq, "kdw": kdw,
         "mask": mask, "trow": trow, "woT": woT}
        for b in range(NCORES)
    ]

    import time as _time

    _t0 = _time.time()
    try:
        res = bass_utils.run_bass_kernel_spmd(
            nc,
            in_maps,
            core_ids=list(range(NCORES)),
            trace=bool(int(os.environ.get("KERNEL_TRACE", "0"))),
        )
    except ModuleNotFoundError:
        res = bass_utils.run_bass_kernel_spmd(
            nc, in_maps, core_ids=list(range(NCORES)), trace=False
        )
    global last_exec_ns
    last_exec_ns = res.exec_time_ns or int((_time.time() - _t0) * 1e9)
    if res.exec_time_ns is not None:
        print(f"HW exec time: {res.exec_time_ns} ns")

    out = np.empty((B, DIM, HW), np.float32)
    for b in range(B):
        q = np.asarray(res.results[b]["out"], np.float32).reshape(DIM, HW // 512, 512)
        s = np.asarray(res.results[b]["osc"], np.float32)
        out[b] = (q * s[:, :, None]).reshape(DIM, HW)
    return np.ascontiguousarray(out.reshape(B, DIM, H, W))
